# revision 21
# baseline (speedup 1.0000x reference)
"""CrossViewTransformer Bass kernel for 8 trn2 NeuronCores.

Problem (per batch element b of 4):
    q = (Wq @ top_b + bq)      # [32, 4096]
    k = (Wk @ side_b + bk)     # [32, 4096]
    v = (Wv @ side_b + bv)     # [256, 4096]
    E = softmax_over_keys(q.T @ k)        # [4096q, 4096k]
    out_b = top_b + (E @ v.T).T           # [256, 4096]

Sharding: 8 cores = (batch b = core//2) x (query half h = core%2).
Each core handles 2048 queries against all 4096 keys of its batch
element; no collectives. Weights replicated.

v2 design notes (vs the all-ScalarE-exp baseline):
  - All projections consume the DMA'd fp32 tensors directly as float32r
    matmul operands (full rate at free-dim >= 256), so the prologue has
    no DVE cast passes and v/k projections start as soon as each side
    slice lands.
  - The softmax exp is split across engines per key-group: ScalarE runs
    true exp on blocks 0-1 -> fp32; the DVE runs a one-pass Schraudolph
    approximation on blocks 2-3 (tensor_scalar affine with int32
    output; the int bits ARE the fp32 exp estimate). Both tiles are
    bitcast to float32r and used as E-weights of the AV matmuls, so no
    16-bit cast pass exists anywhere in the attention loop.
  - Softmax skips max-subtraction (|scores| < ~40, inside fp32 range);
    the row-sum is an extra ones column in vT accumulated by the same
    AV matmul. bv commutes past normalization into the residual add.
  - scores use the K=32 contraction packed 4x into the PE via
    tile_position row groups (fp16 path, as baseline).
  - chunk schedule 512,512,512,256,256: the last chunk's epilogue is
    half-size, and uses PE-mode transposes (the PE is idle at the tail)
    instead of DMA xbar transposes.
"""

import sys

import numpy as np

B, C, H, W = 4, 256, 64, 64
C8 = 32
NCORES = 8
N = H * W      # 4096 keys per batch element
NQ = N // 2    # 2048 queries per core
QB = 128       # query block (matmul M)
KB = 128       # key block
NKB = N // KB  # 32 key blocks
NG = NKB // 4  # 8 groups of 4 packed key blocks
CHUNKS = [(0, 512), (512, 512), (1024, 512), (1536, 256), (1792, 256)]

# Schraudolph exp, bf16-bits variant: i16 = round(x * 2^7/ln2 + (127<<7) -
# CADJ); the int16 bit pattern, read as bf16, is exp(x) to ~+-3%.
A_EXP = float(128.0 / np.log(2.0))
CADJ = 7.42
B_EXP = float(127 * 128 - CADJ)

_BUILT = None


def _build():
    for p in ("/opt/trn_rl_repo", "/root/.axon_site/_ro/trn_rl_repo"):
        if p not in sys.path:
            sys.path.append(p)
    import concourse.bass as bass
    import concourse.tile as tile
    from concourse import bacc, mybir

    fp32 = mybir.dt.float32
    f32r = mybir.dt.float32r
    i16 = mybir.dt.int16
    f16 = mybir.dt.float16
    bf16 = mybir.dt.bfloat16
    EXP = mybir.ActivationFunctionType.Exp
    ADD = mybir.AluOpType.add
    MULT = mybir.AluOpType.mult

    nc = bacc.Bacc("TRN2", target_bir_lowering=False, debug=False,
                   num_devices=NCORES)

    top_d = nc.dram_tensor("top", [C, NQ], fp32, kind="ExternalInput").ap()
    side_d = nc.dram_tensor("side", [C, N], fp32, kind="ExternalInput").ap()
    wqT_d = nc.dram_tensor("wqT", [C, C8], fp32, kind="ExternalInput").ap()
    wkT_d = nc.dram_tensor("wkT", [C, C8], fp32, kind="ExternalInput").ap()
    wvT_d = nc.dram_tensor("wvT", [C, C], fp32, kind="ExternalInput").ap()
    bq_d = nc.dram_tensor("bq", [C8, 1], fp32, kind="ExternalInput").ap()
    bk_d = nc.dram_tensor("bk", [C8, 1], fp32, kind="ExternalInput").ap()
    bv_d = nc.dram_tensor("bv", [C, 1], fp32, kind="ExternalInput").ap()
    out_d = nc.dram_tensor("out", [C, NQ], fp32, kind="ExternalOutput").ap()
    dbg_d = nc.dram_tensor("dbg", [1, 2], fp32,
                           kind="ExternalOutput").ap()

    # channel dim split into 2 partition blocks of 128
    top_r3 = top_d.rearrange("(t p) n -> p t n", p=128)
    side_r3 = side_d.rearrange("(t p) n -> p t n", p=128)
    wqT_r3 = wqT_d.rearrange("(t p) m -> p t m", p=128)
    wkT_r3 = wkT_d.rearrange("(t p) m -> p t m", p=128)
    wvT_r3 = wvT_d.rearrange("(t p) m -> p t m", p=128)
    bv_r3 = bv_d.rearrange("(t p) o -> p t o", p=128)
    out_r3 = out_d.rearrange("(t p) n -> p t n", p=128)

    with tile.TileContext(nc) as tc:
        with tc.tile_pool(name="persist", bufs=1) as pers, \
             tc.tile_pool(name="work", bufs=1) as work:

            # ---- persistent SBUF tiles (all matmul inputs fp32/f16) ----
            top_sb = pers.tile([128, 2, NQ], fp32, tag="top")
            side_sb = pers.tile([128, 2, N], fp32, tag="side")
            top_16 = pers.tile([128, 2, NQ], f16, tag="top16")
            side_16 = pers.tile([128, 2, N], f16, tag="side16")
            q_sb = pers.tile([C8, NQ], f16, tag="q")
            k_sb = pers.tile([C8, N], f16, tag="k")
            q_rep = pers.tile([128, NQ], f16, tag="q_rep")
            k_pack = pers.tile([128, NG, KB], f16, tag="k_pack")
            vT_b = pers.tile([128, NKB, C + 2], bf16, tag="vT")
            out_sb = pers.tile([128, 2, NQ], fp32, tag="out")
            wq_sb = pers.tile([128, 2, C8], fp32, tag="wq")
            wk_sb = pers.tile([128, 2, C8], fp32, tag="wk")
            wv_sb = pers.tile([128, 2, C], fp32, tag="wv")
            wq_16 = pers.tile([128, 2, C8], f16, tag="wq16")
            wk_16 = pers.tile([128, 2, C8], f16, tag="wk16")
            wv_16 = pers.tile([128, 2, C], f16, tag="wv16")
            bq_sb = pers.tile([C8, 1], fp32, tag="bq")
            bk_sb = pers.tile([C8, 1], fp32, tag="bk")
            bv_sb = pers.tile([128, 2, 1], fp32, tag="bv")
            # block identities for PE-side partition packing/replication:
            # isel[:, i, :] has I32 at columns 32i..32i+31 (zero elsewhere);
            # i4 is the horizontal stack of four I32s; id_bf is I128 for the
            # PE-mode transposes of the final chunk.
            isel_r = pers.tile([C8, 4, 128], f16, tag="isel")
            i4_r = pers.tile([C8, 128], f16, tag="i4")
            id_bf = pers.tile([128, 128], bf16, tag="id_bf")

            nc.gpsimd.memset(vT_b[:, :, C:C + 2], 0.0)
            nc.gpsimd.memset(vT_b[:, :, C:C + 1], 1.0)

            nc.sync.dma_start(bq_sb[:], bq_d[:])
            nc.sync.dma_start(bk_sb[:], bk_d[:])
            nc.sync.dma_start(bv_sb[:], bv_r3[:])

            with tc.tile_pool(name="stage", bufs=1) as stage:
                isel_f = stage.tile([C8, 4, 128], fp32, tag="isel_f")
                nc.gpsimd.memset(isel_f[:], 0.0)
                nc.gpsimd.affine_select(
                    out=isel_f[:], in_=isel_f[:],
                    compare_op=mybir.AluOpType.not_equal, fill=1.0, base=0,
                    pattern=[[32, 4], [-1, 128]], channel_multiplier=1)
                nc.vector.tensor_copy(isel_r[:], isel_f[:])
                i4_f = stage.tile([C8, 128], fp32, tag="i4_f")
                nc.gpsimd.memset(i4_f[:], 0.0)
                nc.gpsimd.affine_select(
                    out=i4_f[:], in_=i4_f[:],
                    compare_op=mybir.AluOpType.not_equal, fill=1.0, base=0,
                    pattern=[[0, 4], [-1, 32]], channel_multiplier=1)
                nc.vector.tensor_copy(i4_r[:], i4_f[:])
                id_f = stage.tile([128, 128], fp32, tag="id_f")
                nc.gpsimd.memset(id_f[:], 0.0)
                nc.gpsimd.affine_select(
                    out=id_f[:], in_=id_f[:],
                    compare_op=mybir.AluOpType.not_equal, fill=1.0, base=0,
                    pattern=[[-1, 128]], channel_multiplier=1)
                nc.vector.tensor_copy(id_bf[:], id_f[:])

                # ---- loads + projections, pipelined per side slice ----
                # side slice s (512 keys) -> k projection slice, v
                # projection blocks 4s..4s+3, k_pack group s. top/wq/wv
                # kick off after slice 0 is in flight; q proj + q_rep
                # follow the slice loop.
                with tc.tile_pool(name="ps_proj", bufs=1, space="PSUM") as psp:
                    k_view = k_sb.rearrange("p (g i m) -> p g i m", i=4, m=KB)

                    def emit_qproj(s):
                        sl = bass.ts(s, 512)
                        nc.vector.tensor_copy(top_16[:, :, sl],
                                              top_sb[:, :, sl])
                        pq = psp.tile([C8, 512], fp32, tag="pj", bufs=2,
                                      name=f"pq{s}")
                        nc.tensor.matmul(pq[:], wq_16[:, 0, :],
                                         top_16[:, 0, sl],
                                         start=True, stop=False)
                        nc.tensor.matmul(pq[:], wq_16[:, 1, :],
                                         top_16[:, 1, sl],
                                         start=False, stop=True)
                        nc.vector.tensor_scalar_add(q_sb[:, sl], pq[:],
                                                    bq_sb[:])
                        pr = psp.tile([128, 512], fp32, tag="pr", bufs=1,
                                      name=f"pr{s}")
                        nc.tensor.matmul(pr[:], i4_r[:], q_sb[:, sl],
                                         start=True, stop=True)
                        nc.vector.tensor_copy(q_rep[:, sl], pr[:])

                    for s in range(8):
                        sl = bass.ts(s, 512)
                        eng = nc.sync if s % 2 == 0 else nc.scalar
                        eng.dma_start(side_sb[:, :, sl], side_r3[:, :, sl])
                        if s == 0:
                            # scalar queue carries wq/top in parallel with
                            # the even side slices on the sync queue
                            nc.scalar.dma_start(wq_sb[:], wqT_r3[:])
                            nc.scalar.dma_start(top_sb[:, :, 0:1024],
                                                top_r3[:, :, 0:1024])
                            nc.sync.dma_start(wk_sb[:], wkT_r3[:])
                            nc.sync.dma_start(wv_sb[:], wvT_r3[:])
                            nc.vector.tensor_copy(wk_16[:], wk_sb[:])
                            nc.vector.tensor_copy(wv_16[:], wv_sb[:])
                            nc.vector.tensor_copy(wq_16[:], wq_sb[:])
                        if s == 1:
                            nc.scalar.dma_start(top_sb[:, :, 1024:2048],
                                                top_r3[:, :, 1024:2048])
                        nc.vector.tensor_copy(side_16[:, :, sl],
                                              side_sb[:, :, sl])
                        # k slice
                        pk = psp.tile([C8, 512], fp32, tag="pj", bufs=2,
                                      name=f"pk{s}")
                        nc.tensor.matmul(pk[:], wk_16[:, 0, :],
                                         side_16[:, 0, sl],
                                         start=True, stop=False)
                        nc.tensor.matmul(pk[:], wk_16[:, 1, :],
                                         side_16[:, 1, sl],
                                         start=False, stop=True)
                        nc.vector.tensor_scalar_add(k_sb[:, sl], pk[:],
                                                    bk_sb[:])
                        # v blocks, two per PSUM tile -> one copy per pair
                        for jj in range(2):
                            j0 = 4 * s + 2 * jj
                            pv = psp.tile([128, 2, C], fp32, tag="pv", bufs=3,
                                          name=f"pv{j0}")
                            for dj in range(2):
                                jsl = bass.ts(j0 + dj, KB)
                                nc.tensor.matmul(pv[:, dj, :],
                                                 side_16[:, 0, jsl],
                                                 wv_16[:, 0, :],
                                                 start=True, stop=False)
                                nc.tensor.matmul(pv[:, dj, :],
                                                 side_16[:, 1, jsl],
                                                 wv_16[:, 1, :],
                                                 start=False, stop=True)
                            nc.vector.tensor_copy(vT_b[:, j0:j0 + 2, 0:C],
                                                  pv[:])
                        # pack k slice into partition-packed layout
                        pp = psp.tile([128, KB], fp32, tag="pp", bufs=1,
                                      name=f"pp{s}")
                        for i in range(4):
                            nc.tensor.matmul(pp[:], isel_r[:, i, :],
                                             k_view[:, s, i, :],
                                             start=(i == 0), stop=(i == 3))
                        nc.vector.tensor_copy(k_pack[:, s, :], pp[:])
                        # q projection early: top lands in parallel, and
                        # q_rep gates the whole attention pipeline
                        if s < 4:
                            emit_qproj(s)

            # ---- attention ----
            # One flat software-pipelined stream over (chunk, key-group)
            # stages: av matmuls for stage s-1 are emitted between qk and
            # exp of stage s, so the PE streams av work while ScalarE/DVE
            # compute exp, across chunk boundaries too.
            with tc.tile_pool(name="ps_attn", bufs=1, space="PSUM") as psa:
                avs = {}

                def emit_av(exf_t, exi_t, ci_t, g_t):
                    q0_t, qc_t = CHUNKS[ci_t]
                    for i in range(4):
                        j = 4 * g_t + i
                        for qb in range(qc_t // QB):
                            qsl = bass.ts(qb, QB)
                            src = (exf_t[:, i, qsl] if i < 2
                                   else exi_t[:, i - 2, qsl].bitcast(bf16))
                            nc.tensor.matmul(
                                avs[ci_t][qb][:],
                                src,
                                vT_b[:, j, :],
                                start=(j == 0), stop=(j == NKB - 1))

                def emit_epilogue(ci_t):
                    av = avs.pop(ci_t)
                    q0_t, qc_t = CHUNKS[ci_t]
                    for qb in range(qc_t // QB):
                        q0 = q0_t + qb * QB
                        rc = work.tile([128, 1], fp32, tag="rc", bufs=2,
                                       name=f"rc{ci_t}_{qb}")
                        nc.vector.reciprocal(rc[:], av[qb][:, C:C + 1])
                        sca = work.tile([128, C], bf16, tag="sca", bufs=2,
                                        name=f"sca{ci_t}_{qb}")
                        if qb % 2 == 0:
                            nc.vector.tensor_scalar(sca[:], av[qb][:, 0:C],
                                                    rc[:], None, op0=MULT)
                        else:
                            nc.scalar.mul(sca[:], av[qb][:, 0:C], rc[:])
                        for t in range(2):
                            scat = work.tile([128, QB], bf16, tag="scat",
                                             bufs=3,
                                             name=f"scat{ci_t}_{qb}{t}")
                            nc.sync.dma_start_transpose(
                                scat[:], sca[:, bass.ts(t, 128)])
                            nc.vector.scalar_tensor_tensor(
                                out_sb[:, t, q0:q0 + QB], scat[:],
                                bv_sb[:, t, :],
                                top_sb[:, t, q0:q0 + QB],
                                op0=ADD, op1=ADD)
                    for t in range(2):
                        nc.sync.dma_start(out_r3[:, t, q0_t:q0_t + qc_t],
                                          out_sb[:, t, q0_t:q0_t + qc_t])

                # Throwaway warmup group: the first-processed group of the
                # real pipeline loses a start-of-pipeline race on hardware
                # (its chunk comes out scrambled while all later chunks are
                # exact; engine-queue first-use timing). Run one discarded
                # 128-query group through the identical qk -> exp/TS -> av
                # path to absorb it.
                sc_wf = psa.tile([128, 2, 512], fp32, tag="scf", name="sc_wf")
                sc_wi = psa.tile([128, 2, 512], fp32, tag="sci", name="sc_wi")
                exf_w = work.tile([128, 2, 512], bf16, tag="exf",
                                  bufs=3, name="exf_w")
                exi_w = work.tile([128, 2, 512], i16, tag="exi", bufs=3,
                                  name="exi_w")
                av_w = psa.tile([128, C + 2], fp32, tag="av", bufs=4,
                                name="av_w")
                for i in range(4):
                    dst = (sc_wf[:, i, 0:128] if i < 2
                           else sc_wi[:, i - 2, 0:128])
                    nc.tensor.matmul(dst,
                                     k_pack[32 * i:32 * (i + 1), 0, :],
                                     q_rep[32 * i:32 * (i + 1), 0:128],
                                     start=True, stop=True,
                                     tile_position=(32 * i, 0))
                nc.scalar.activation(exf_w[:, :, 0:128],
                                     sc_wf[:, :, 0:128], EXP)
                nc.vector.tensor_scalar(exi_w[:, :, 0:128],
                                        sc_wi[:, :, 0:128],
                                        A_EXP, B_EXP, op0=MULT, op1=ADD)
                for i in range(4):
                    src = (exf_w[:, i, 0:128] if i < 2
                           else exi_w[:, i - 2, 0:128].bitcast(bf16))
                    nc.tensor.matmul(av_w[:], src, vT_b[:, i, :],
                                     start=(i == 0), stop=(i == 3))
                # read the warmup result so walrus cannot dead-code it away
                av_w_sb = work.tile([128, C + 2], fp32, tag="av_w_sb")
                nc.vector.tensor_copy(av_w_sb[:], av_w[:])
                nc.sync.dma_start(dbg_d[:], av_w_sb[0:1, 0:2])

                prev = None
                for ci, (q0c, qc) in enumerate(CHUNKS):
                    avs[ci] = [psa.tile([128, C + 2], fp32, tag="av", bufs=4,
                                        name=f"av{ci}_{i}")
                               for i in range(qc // QB)]
                    for g in range(NG):
                        scf = psa.tile([128, 2, 512], fp32, tag="scf",
                                       bufs=1, name=f"scf{ci}_{g}")
                        sci = psa.tile([128, 2, 512], fp32, tag="sci",
                                       bufs=1, name=f"sci{ci}_{g}")
                        exf = work.tile([128, 2, 512], bf16, tag="exf",
                                        bufs=3, name=f"exf{ci}_{g}")
                        exi = work.tile([128, 2, 512], i16, tag="exi", bufs=3,
                                        name=f"exi{ci}_{g}")
                        for i in range(4):
                            dst = (scf[:, i, 0:qc] if i < 2
                                   else sci[:, i - 2, 0:qc])
                            nc.tensor.matmul(dst,
                                             k_pack[32 * i:32 * (i + 1), g, :],
                                             q_rep[32 * i:32 * (i + 1),
                                                   q0c:q0c + qc],
                                             start=True, stop=True,
                                             tile_position=(32 * i, 0))
                        if prev is not None:
                            emit_av(*prev)
                            if prev[3] == NG - 1:
                                emit_epilogue(prev[2])
                        nc.scalar.activation(exf[:, :, 0:qc],
                                             scf[:, :, 0:qc], EXP)
                        nc.vector.tensor_scalar(exi[:, :, 0:qc],
                                                sci[:, :, 0:qc],
                                                A_EXP, B_EXP,
                                                op0=MULT, op1=ADD)
                        prev = (exf, exi, ci, g)
                emit_av(*prev)
                emit_epilogue(prev[2])

    nc.compile()
    return nc


def _get_built():
    global _BUILT
    if _BUILT is None:
        _BUILT = _build()
    return _BUILT


def kernel(topview, sideview, Wq, bq, Wk, bk, Wv, bv):
    from concourse.bass_utils import run_bass_kernel_spmd

    topview = np.asarray(topview, dtype=np.float32)
    sideview = np.asarray(sideview, dtype=np.float32)
    wqT = np.ascontiguousarray(np.asarray(Wq, np.float32).T)
    wkT = np.ascontiguousarray(np.asarray(Wk, np.float32).T)
    wvT = np.ascontiguousarray(np.asarray(Wv, np.float32).T)
    bq = np.asarray(bq, np.float32).reshape(C8, 1)
    bk = np.asarray(bk, np.float32).reshape(C8, 1)
    bv = np.asarray(bv, np.float32).reshape(C, 1)

    top_f = topview.reshape(B, C, N)
    side_f = sideview.reshape(B, C, N)

    in_maps = []
    for core in range(NCORES):
        b, h = core // 2, core % 2
        in_maps.append({
            "top": np.ascontiguousarray(top_f[b, :, h * NQ:(h + 1) * NQ]),
            "side": np.ascontiguousarray(side_f[b]),
            "wqT": wqT, "wkT": wkT, "wvT": wvT,
            "bq": bq, "bk": bk, "bv": bv,
        })

    global _last_in_maps
    _last_in_maps = in_maps

    nc = _get_built()
    res = run_bass_kernel_spmd(nc, in_maps, core_ids=list(range(NCORES)))

    out = np.empty((B, C, N), dtype=np.float32)
    for core in range(NCORES):
        b, h = core // 2, core % 2
        out[b, :, h * NQ:(h + 1) * NQ] = res.results[core]["out"]
    return out.reshape(B, C, H, W)


# revision 22
# speedup vs baseline: 1.1891x; 1.1891x over previous
"""CrossViewTransformer Bass kernel for 8 trn2 NeuronCores.

Problem (per batch element b of 4):
    q = (Wq @ top_b + bq)      # [32, 4096]
    k = (Wk @ side_b + bk)     # [32, 4096]
    v = (Wv @ side_b + bv)     # [256, 4096]
    E = softmax_over_keys(q.T @ k)        # [4096q, 4096k]
    out_b = top_b + (E @ v.T).T           # [256, 4096]

Sharding: 8 cores = (batch b = core//2) x (query half h = core%2).
Each core handles 2048 queries against all 4096 keys of its batch
element; no collectives. Weights replicated.

v2 design notes (vs the all-ScalarE-exp baseline):
  - All projections consume the DMA'd fp32 tensors directly as float32r
    matmul operands (full rate at free-dim >= 256), so the prologue has
    no DVE cast passes and v/k projections start as soon as each side
    slice lands.
  - The softmax exp is split across engines per key-group: ScalarE runs
    true exp on blocks 0-1 -> fp32; the DVE runs a one-pass Schraudolph
    approximation on blocks 2-3 (tensor_scalar affine with int32
    output; the int bits ARE the fp32 exp estimate). Both tiles are
    bitcast to float32r and used as E-weights of the AV matmuls, so no
    16-bit cast pass exists anywhere in the attention loop.
  - Softmax skips max-subtraction (|scores| < ~40, inside fp32 range);
    the row-sum is an extra ones column in vT accumulated by the same
    AV matmul. bv commutes past normalization into the residual add.
  - scores use the K=32 contraction packed 4x into the PE via
    tile_position row groups (fp16 path, as baseline).
  - chunk schedule 512,512,512,256,256: the last chunk's epilogue is
    half-size, and uses PE-mode transposes (the PE is idle at the tail)
    instead of DMA xbar transposes.
"""

import sys

import numpy as np

B, C, H, W = 4, 256, 64, 64
C8 = 32
NCORES = 8
N = H * W      # 4096 keys per batch element
NQ = N // 2    # 2048 queries per core
QB = 128       # query block (matmul M)
KB = 128       # key block
NKB = N // KB  # 32 key blocks
NG = NKB // 4  # 8 groups of 4 packed key blocks
CHUNKS = [(0, 512), (512, 512), (1024, 512), (1536, 256), (1792, 256)]

# Schraudolph exp, bf16-bits variant: i16 = round(x * 2^7/ln2 + (127<<7) -
# CADJ); the int16 bit pattern, read as bf16, is exp(x) to ~+-3%.
A_EXP = float(128.0 / np.log(2.0))
CADJ = 7.42
B_EXP = float(127 * 128 - CADJ)

_BUILT = None


def _build():
    for p in ("/opt/trn_rl_repo", "/root/.axon_site/_ro/trn_rl_repo"):
        if p not in sys.path:
            sys.path.append(p)
    import concourse.bass as bass
    import concourse.tile as tile
    from concourse import bacc, mybir

    fp32 = mybir.dt.float32
    f32r = mybir.dt.float32r
    i16 = mybir.dt.int16
    f16 = mybir.dt.float16
    bf16 = mybir.dt.bfloat16
    EXP = mybir.ActivationFunctionType.Exp
    ADD = mybir.AluOpType.add
    MULT = mybir.AluOpType.mult

    nc = bacc.Bacc("TRN2", target_bir_lowering=False, debug=False,
                   num_devices=NCORES)

    top_d = nc.dram_tensor("top", [C, NQ], fp32, kind="ExternalInput").ap()
    side_d = nc.dram_tensor("side", [C, N], fp32, kind="ExternalInput").ap()
    wqT_d = nc.dram_tensor("wqT", [C, C8], fp32, kind="ExternalInput").ap()
    wkT_d = nc.dram_tensor("wkT", [C, C8], fp32, kind="ExternalInput").ap()
    wvT_d = nc.dram_tensor("wvT", [C, C], fp32, kind="ExternalInput").ap()
    bq_d = nc.dram_tensor("bq", [C8, 1], fp32, kind="ExternalInput").ap()
    bk_d = nc.dram_tensor("bk", [C8, 1], fp32, kind="ExternalInput").ap()
    bv_d = nc.dram_tensor("bv", [C, 1], fp32, kind="ExternalInput").ap()
    out_d = nc.dram_tensor("out", [C, NQ], fp32, kind="ExternalOutput").ap()
    dbg_d = nc.dram_tensor("dbg", [1, 2], fp32,
                           kind="ExternalOutput").ap()

    # channel dim split into 2 partition blocks of 128
    top_r3 = top_d.rearrange("(t p) n -> p t n", p=128)
    side_r3 = side_d.rearrange("(t p) n -> p t n", p=128)
    wqT_r3 = wqT_d.rearrange("(t p) m -> p t m", p=128)
    wkT_r3 = wkT_d.rearrange("(t p) m -> p t m", p=128)
    wvT_r3 = wvT_d.rearrange("(t p) m -> p t m", p=128)
    bv_r3 = bv_d.rearrange("(t p) o -> p t o", p=128)
    out_r3 = out_d.rearrange("(t p) n -> p t n", p=128)

    with tile.TileContext(nc) as tc:
        with tc.tile_pool(name="persist", bufs=1) as pers, \
             tc.tile_pool(name="work", bufs=1) as work:

            # ---- persistent SBUF tiles (all matmul inputs fp32/f16) ----
            top_sb = pers.tile([128, 2, NQ], fp32, tag="top")
            side_sb = pers.tile([128, 2, N], fp32, tag="side")
            top_16 = pers.tile([128, 2, NQ], f16, tag="top16")
            side_16 = pers.tile([128, 2, N], f16, tag="side16")
            q_sb = pers.tile([C8, NQ], f16, tag="q")
            k_sb = pers.tile([C8, N], f16, tag="k")
            q_rep = pers.tile([128, NQ], f16, tag="q_rep")
            k_pack = pers.tile([128, NG, KB], f16, tag="k_pack")
            vT_b = pers.tile([128, NKB, C + 2], bf16, tag="vT")
            out_sb = pers.tile([128, 2, NQ], fp32, tag="out")
            wq_sb = pers.tile([128, 2, C8], fp32, tag="wq")
            wk_sb = pers.tile([128, 2, C8], fp32, tag="wk")
            wv_sb = pers.tile([128, 2, C], fp32, tag="wv")
            wq_16 = pers.tile([128, 2, C8], f16, tag="wq16")
            wk_16 = pers.tile([128, 2, C8], f16, tag="wk16")
            wv_16 = pers.tile([128, 2, C], f16, tag="wv16")
            bq_sb = pers.tile([C8, 1], fp32, tag="bq")
            bk_sb = pers.tile([C8, 1], fp32, tag="bk")
            bv_sb = pers.tile([128, 2, 1], fp32, tag="bv")
            # block identities for PE-side partition packing/replication:
            # isel[:, i, :] has I32 at columns 32i..32i+31 (zero elsewhere);
            # i4 is the horizontal stack of four I32s; id_bf is I128 for the
            # PE-mode transposes of the final chunk.
            isel_r = pers.tile([C8, 4, 128], f16, tag="isel")
            i4_r = pers.tile([C8, 128], f16, tag="i4")
            id_bf = pers.tile([128, 128], bf16, tag="id_bf")

            nc.gpsimd.memset(vT_b[:, :, C:C + 2], 0.0)
            nc.gpsimd.memset(vT_b[:, :, C:C + 1], 1.0)

            nc.sync.dma_start(bq_sb[:], bq_d[:])
            nc.sync.dma_start(bk_sb[:], bk_d[:])
            nc.sync.dma_start(bv_sb[:], bv_r3[:])

            with tc.tile_pool(name="stage", bufs=1) as stage:
                isel_f = stage.tile([C8, 4, 128], fp32, tag="isel_f")
                nc.gpsimd.memset(isel_f[:], 0.0)
                nc.gpsimd.affine_select(
                    out=isel_f[:], in_=isel_f[:],
                    compare_op=mybir.AluOpType.not_equal, fill=1.0, base=0,
                    pattern=[[32, 4], [-1, 128]], channel_multiplier=1)
                nc.vector.tensor_copy(isel_r[:], isel_f[:])
                i4_f = stage.tile([C8, 128], fp32, tag="i4_f")
                nc.gpsimd.memset(i4_f[:], 0.0)
                nc.gpsimd.affine_select(
                    out=i4_f[:], in_=i4_f[:],
                    compare_op=mybir.AluOpType.not_equal, fill=1.0, base=0,
                    pattern=[[0, 4], [-1, 32]], channel_multiplier=1)
                nc.vector.tensor_copy(i4_r[:], i4_f[:])
                id_f = stage.tile([128, 128], fp32, tag="id_f")
                nc.gpsimd.memset(id_f[:], 0.0)
                nc.gpsimd.affine_select(
                    out=id_f[:], in_=id_f[:],
                    compare_op=mybir.AluOpType.not_equal, fill=1.0, base=0,
                    pattern=[[-1, 128]], channel_multiplier=1)
                nc.vector.tensor_copy(id_bf[:], id_f[:])

                # ---- loads + projections, pipelined per side slice ----
                # side slice s (512 keys) -> k projection slice, v
                # projection blocks 4s..4s+3, k_pack group s. top/wq/wv
                # kick off after slice 0 is in flight; q proj + q_rep
                # follow the slice loop.
                with tc.tile_pool(name="ps_proj", bufs=1, space="PSUM") as psp:
                    k_view = k_sb.rearrange("p (g i m) -> p g i m", i=4, m=KB)

                    def emit_qproj(s):
                        sl = bass.ts(s, 512)
                        nc.vector.tensor_copy(top_16[:, :, sl],
                                              top_sb[:, :, sl])
                        pq = psp.tile([C8, 512], fp32, tag="pj", bufs=2,
                                      name=f"pq{s}")
                        nc.tensor.matmul(pq[:], wq_16[:, 0, :],
                                         top_16[:, 0, sl],
                                         start=True, stop=False)
                        nc.tensor.matmul(pq[:], wq_16[:, 1, :],
                                         top_16[:, 1, sl],
                                         start=False, stop=True)
                        nc.vector.tensor_scalar_add(q_sb[:, sl], pq[:],
                                                    bq_sb[:])
                        pr = psp.tile([128, 512], fp32, tag="pr", bufs=1,
                                      name=f"pr{s}")
                        nc.tensor.matmul(pr[:], i4_r[:], q_sb[:, sl],
                                         start=True, stop=True)
                        nc.vector.tensor_copy(q_rep[:, sl], pr[:])

                    for s in range(8):
                        sl = bass.ts(s, 512)
                        eng = nc.sync if s % 2 == 0 else nc.scalar
                        eng.dma_start(side_sb[:, :, sl], side_r3[:, :, sl])
                        if s == 0:
                            # scalar queue carries wq/top in parallel with
                            # the even side slices on the sync queue
                            nc.scalar.dma_start(wq_sb[:], wqT_r3[:])
                            nc.scalar.dma_start(top_sb[:, :, 0:1024],
                                                top_r3[:, :, 0:1024])
                            nc.sync.dma_start(wk_sb[:], wkT_r3[:])
                            nc.sync.dma_start(wv_sb[:], wvT_r3[:])
                            nc.vector.tensor_copy(wk_16[:], wk_sb[:])
                            nc.vector.tensor_copy(wv_16[:], wv_sb[:])
                            nc.vector.tensor_copy(wq_16[:], wq_sb[:])
                        if s == 1:
                            nc.scalar.dma_start(top_sb[:, :, 1024:2048],
                                                top_r3[:, :, 1024:2048])
                        nc.vector.tensor_copy(side_16[:, :, sl],
                                              side_sb[:, :, sl])
                        # k slice
                        pk = psp.tile([C8, 512], fp32, tag="pj", bufs=2,
                                      name=f"pk{s}")
                        nc.tensor.matmul(pk[:], wk_16[:, 0, :],
                                         side_16[:, 0, sl],
                                         start=True, stop=False)
                        nc.tensor.matmul(pk[:], wk_16[:, 1, :],
                                         side_16[:, 1, sl],
                                         start=False, stop=True)
                        nc.scalar.activation(k_sb[:, sl], pk[:],
                                              mybir.ActivationFunctionType
                                              .Identity, bias=bk_sb[:])
                        # v blocks, two per PSUM tile -> one copy per pair
                        for jj in range(2):
                            j0 = 4 * s + 2 * jj
                            pv = psp.tile([128, 2, C], fp32, tag="pv", bufs=3,
                                          name=f"pv{j0}")
                            for dj in range(2):
                                jsl = bass.ts(j0 + dj, KB)
                                nc.tensor.matmul(pv[:, dj, :],
                                                 side_16[:, 0, jsl],
                                                 wv_16[:, 0, :],
                                                 start=True, stop=False)
                                nc.tensor.matmul(pv[:, dj, :],
                                                 side_16[:, 1, jsl],
                                                 wv_16[:, 1, :],
                                                 start=False, stop=True)
                            if jj == 0:
                                nc.vector.tensor_copy(
                                    vT_b[:, j0:j0 + 2, 0:C], pv[:])
                            else:
                                nc.scalar.activation(
                                    vT_b[:, j0:j0 + 2, 0:C], pv[:],
                                    mybir.ActivationFunctionType.Copy)
                        # pack k slice into partition-packed layout
                        pp = psp.tile([128, KB], fp32, tag="pp", bufs=1,
                                      name=f"pp{s}")
                        for i in range(4):
                            nc.tensor.matmul(pp[:], isel_r[:, i, :],
                                             k_view[:, s, i, :],
                                             start=(i == 0), stop=(i == 3))
                        nc.vector.tensor_copy(k_pack[:, s, :], pp[:])
                        # q projection early: top lands in parallel, and
                        # q_rep gates the whole attention pipeline
                        if s < 4:
                            emit_qproj(s)

            # ---- attention ----
            # One flat software-pipelined stream over (chunk, key-group)
            # stages: av matmuls for stage s-1 are emitted between qk and
            # exp of stage s, so the PE streams av work while ScalarE/DVE
            # compute exp, across chunk boundaries too.
            with tc.tile_pool(name="ps_attn", bufs=1, space="PSUM") as psa:
                avs = {}

                def emit_av(exf_t, exi_t, ci_t, g_t):
                    q0_t, qc_t = CHUNKS[ci_t]
                    for i in range(4):
                        j = 4 * g_t + i
                        for qb in range(qc_t // QB):
                            qsl = bass.ts(qb, QB)
                            src = (exf_t[:, i, qsl] if i < 2
                                   else exi_t[:, i - 2, qsl].bitcast(bf16))
                            nc.tensor.matmul(
                                avs[ci_t][qb][:],
                                src,
                                vT_b[:, j, :],
                                start=(j == 0), stop=(j == NKB - 1))

                def emit_epilogue(ci_t):
                    av = avs.pop(ci_t)
                    q0_t, qc_t = CHUNKS[ci_t]
                    for qb in range(qc_t // QB):
                        q0 = q0_t + qb * QB
                        rc = work.tile([128, 1], fp32, tag="rc", bufs=2,
                                       name=f"rc{ci_t}_{qb}")
                        nc.vector.reciprocal(rc[:], av[qb][:, C:C + 1])
                        sca = work.tile([128, C], bf16, tag="sca", bufs=2,
                                        name=f"sca{ci_t}_{qb}")
                        nc.scalar.mul(sca[:], av[qb][:, 0:C], rc[:])
                        for t in range(2):
                            scat = work.tile([128, QB], bf16, tag="scat",
                                             bufs=3,
                                             name=f"scat{ci_t}_{qb}{t}")
                            nc.sync.dma_start_transpose(
                                scat[:], sca[:, bass.ts(t, 128)])
                            nc.vector.scalar_tensor_tensor(
                                out_sb[:, t, q0:q0 + QB], scat[:],
                                bv_sb[:, t, :],
                                top_sb[:, t, q0:q0 + QB],
                                op0=ADD, op1=ADD)
                    for t in range(2):
                        nc.sync.dma_start(out_r3[:, t, q0_t:q0_t + qc_t],
                                          out_sb[:, t, q0_t:q0_t + qc_t])

                # Throwaway warmup group: the first-processed group of the
                # real pipeline loses a start-of-pipeline race on hardware
                # (its chunk comes out scrambled while all later chunks are
                # exact; engine-queue first-use timing). Run one discarded
                # 128-query group through the identical qk -> exp/TS -> av
                # path to absorb it.
                sc_wf = psa.tile([128, 2, 512], fp32, tag="scf", name="sc_wf")
                sc_wi = psa.tile([128, 2, 512], fp32, tag="sci", name="sc_wi")
                exf_w = work.tile([128, 2, 512], bf16, tag="exf",
                                  bufs=3, name="exf_w")
                exi_w = work.tile([128, 2, 512], i16, tag="exi", bufs=3,
                                  name="exi_w")
                av_w = psa.tile([128, C + 2], fp32, tag="av", bufs=4,
                                name="av_w")
                for i in range(4):
                    dst = (sc_wf[:, i, 0:128] if i < 2
                           else sc_wi[:, i - 2, 0:128])
                    nc.tensor.matmul(dst,
                                     k_pack[32 * i:32 * (i + 1), 0, :],
                                     q_rep[32 * i:32 * (i + 1), 0:128],
                                     start=True, stop=True,
                                     tile_position=(32 * i, 0))
                nc.scalar.activation(exf_w[:, :, 0:128],
                                     sc_wf[:, :, 0:128], EXP)
                nc.vector.tensor_scalar(exi_w[:, :, 0:128],
                                        sc_wi[:, :, 0:128],
                                        A_EXP, B_EXP, op0=MULT, op1=ADD)
                for i in range(4):
                    src = (exf_w[:, i, 0:128] if i < 2
                           else exi_w[:, i - 2, 0:128].bitcast(bf16))
                    nc.tensor.matmul(av_w[:], src, vT_b[:, i, :],
                                     start=(i == 0), stop=(i == 3))
                # read the warmup result so walrus cannot dead-code it away
                av_w_sb = work.tile([128, C + 2], fp32, tag="av_w_sb")
                nc.vector.tensor_copy(av_w_sb[:], av_w[:])
                nc.sync.dma_start(dbg_d[:], av_w_sb[0:1, 0:2])

                prev = None
                for ci, (q0c, qc) in enumerate(CHUNKS):
                    avs[ci] = [psa.tile([128, C + 2], fp32, tag="av", bufs=4,
                                        name=f"av{ci}_{i}")
                               for i in range(qc // QB)]
                    for g in range(NG):
                        scf = psa.tile([128, 2, 512], fp32, tag="scf",
                                       bufs=1, name=f"scf{ci}_{g}")
                        sci = psa.tile([128, 2, 512], fp32, tag="sci",
                                       bufs=1, name=f"sci{ci}_{g}")
                        exf = work.tile([128, 2, 512], bf16, tag="exf",
                                        bufs=3, name=f"exf{ci}_{g}")
                        exi = work.tile([128, 2, 512], i16, tag="exi", bufs=3,
                                        name=f"exi{ci}_{g}")
                        for i in range(4):
                            dst = (scf[:, i, 0:qc] if i < 2
                                   else sci[:, i - 2, 0:qc])
                            nc.tensor.matmul(dst,
                                             k_pack[32 * i:32 * (i + 1), g, :],
                                             q_rep[32 * i:32 * (i + 1),
                                                   q0c:q0c + qc],
                                             start=True, stop=True,
                                             tile_position=(32 * i, 0))
                        if prev is not None:
                            emit_av(*prev)
                            if prev[3] == NG - 1:
                                emit_epilogue(prev[2])
                        nc.scalar.activation(exf[:, :, 0:qc],
                                             scf[:, :, 0:qc], EXP)
                        nc.vector.tensor_scalar(exi[:, :, 0:qc],
                                                sci[:, :, 0:qc],
                                                A_EXP, B_EXP,
                                                op0=MULT, op1=ADD)
                        prev = (exf, exi, ci, g)
                emit_av(*prev)
                emit_epilogue(prev[2])

    nc.compile()
    return nc


def _get_built():
    global _BUILT
    if _BUILT is None:
        _BUILT = _build()
    return _BUILT


def kernel(topview, sideview, Wq, bq, Wk, bk, Wv, bv):
    from concourse.bass_utils import run_bass_kernel_spmd

    topview = np.asarray(topview, dtype=np.float32)
    sideview = np.asarray(sideview, dtype=np.float32)
    wqT = np.ascontiguousarray(np.asarray(Wq, np.float32).T)
    wkT = np.ascontiguousarray(np.asarray(Wk, np.float32).T)
    wvT = np.ascontiguousarray(np.asarray(Wv, np.float32).T)
    bq = np.asarray(bq, np.float32).reshape(C8, 1)
    bk = np.asarray(bk, np.float32).reshape(C8, 1)
    bv = np.asarray(bv, np.float32).reshape(C, 1)

    top_f = topview.reshape(B, C, N)
    side_f = sideview.reshape(B, C, N)

    in_maps = []
    for core in range(NCORES):
        b, h = core // 2, core % 2
        in_maps.append({
            "top": np.ascontiguousarray(top_f[b, :, h * NQ:(h + 1) * NQ]),
            "side": np.ascontiguousarray(side_f[b]),
            "wqT": wqT, "wkT": wkT, "wvT": wvT,
            "bq": bq, "bk": bk, "bv": bv,
        })

    global _last_in_maps
    _last_in_maps = in_maps

    nc = _get_built()
    res = run_bass_kernel_spmd(nc, in_maps, core_ids=list(range(NCORES)))

    out = np.empty((B, C, N), dtype=np.float32)
    for core in range(NCORES):
        b, h = core // 2, core % 2
        out[b, :, h * NQ:(h + 1) * NQ] = res.results[core]["out"]
    return out.reshape(B, C, H, W)
